# revision 12
# baseline (speedup 1.0000x reference)
"""Trainium2 Bass kernel: Conv2d [8,8,1024,1024] x [8,8,3,3] (+bias), with
the reference's roll-by-1 on H, VALID padding -> [8,8,1022,1022].

Strategy: data-parallel over the batch dim (1 image per NeuronCore, 8 cores).
The kernel is HBM-bandwidth bound (and the device HAM throttles HBM to ~50%
when all 8 cores stream), so the wire format is bf16 both ways and the host
pre-relayouts the input so every DMA moves big contiguous per-partition
spans:

  - Host packs the (rolled) input as inp_re[q*8+c, b*1024+w] =
    bf16(inp[c, (14b+q-1)%1024, w]): partition p = (row-in-block q, cin c),
    one 1024-col slab per conv block b (73 blocks x 14 output rows = 1022).
  - Per group of G=8..9 blocks, ONE input DMA moves [128, G*1024] with
    G*2KiB contiguous per partition; compute runs blocked matmuls on the
    tensor engine: lhsT [128,112] bf16 packs filt taps (column m = dx*8+co
    holds filt[co,c,q-dx,j]); the 3 W-taps are accumulating matmuls whose
    rhs is the same tile shifted by j. PSUM f32 [112,512] is evicted by
    DVE tensor_scalar_add(+bias) into a bf16 SBUF tile, and ONE output DMA
    per group writes [112, G*1022] contiguous.
  - Host unshards out_re[dx*8+co, b*1022+w] -> out[co, 14b+dx, w] and
    upcasts to f32.

bf16 error (inputs+weights+output quantized, f32 PSUM accumulate over the
72-term contraction) is ~0.3% of output scale, well under the 2e-2 gate.
"""

import os
import sys

for _p in ("/opt/trn_rl_repo",):
    if _p not in sys.path and os.path.isdir(_p):
        sys.path.insert(0, _p)

import ml_dtypes
import numpy as np

import concourse.bacc as bacc
import concourse.mybir as mybir
from concourse.bass_utils import run_bass_kernel_spmd
from concourse.tile import TileContext

F32 = mybir.dt.float32
BF16 = mybir.dt.bfloat16
NPBF16 = ml_dtypes.bfloat16

N_CORES = 8
CIN = 8
COUT = 8
KH = 3
KW = 3
H = W = 1024
HOUT = WOUT = 1022
D = 14            # output rows per block
R = D + 2         # input rows per block
M = COUT * D      # 112 matmul output columns (dx-major)
MPAD = 128        # lhsT padded to 128 cols: enables PE Fast Weight Load
NB = HOUT // D    # 73 blocks exactly
GROUPS = [1, 1, 2, 3, 4] + [4] * 14 + [3, 2, 1]   # sum = 73; small head+tail
GMAX = max(GROUPS)
CHUNKS = ((0, 512), (512, 510))  # PSUM bank = 512 f32


def build_nc(in_bufs: int = 6, out_bufs: int = 6, psum_bufs: int = 8):
    nc = bacc.Bacc("TRN2", target_bir_lowering=False, debug=False,
                   num_devices=N_CORES)
    inp_d = nc.dram_tensor("inp", [R * CIN, NB * W], BF16,
                           kind="ExternalInput")
    wgt_d = nc.dram_tensor("wgt", [R * CIN, KW * MPAD], BF16,
                           kind="ExternalInput")
    bias_d = nc.dram_tensor("bias", [M, 1], F32, kind="ExternalInput")
    out_d = nc.dram_tensor("out", [M, NB * WOUT], BF16,
                           kind="ExternalOutput")

    with TileContext(nc) as tc:
        with (
            tc.tile_pool(name="consts", bufs=1) as wpool,
            tc.tile_pool(name="inp", bufs=in_bufs) as ipool,
            tc.tile_pool(name="outp", bufs=out_bufs) as opool,
            tc.tile_pool(name="ps", bufs=psum_bufs, space="PSUM") as ppool,
        ):
            ident = mybir.ActivationFunctionType.Identity
            wt = wpool.tile([R * CIN, KW * MPAD], BF16, tag="wgt")
            nc.sync.dma_start(out=wt[:], in_=wgt_d[:])
            bt = wpool.tile([M, 1], F32, tag="bias")
            nc.sync.dma_start(out=bt[:], in_=bias_d[:])

            b0 = 0
            for gi, G in enumerate(GROUPS):
                t = ipool.tile([R * CIN, GMAX * W], BF16, tag="inp")
                # Alternate input DMAs across two HWDGE queues so the
                # descriptor-gen of consecutive groups overlaps.
                ieng = nc.sync if gi % 2 == 0 else nc.scalar
                ieng.dma_start(out=t[:, 0:G * W],
                               in_=inp_d[:, b0 * W:(b0 + G) * W])
                ot = opool.tile([M, GMAX * WOUT], BF16, tag="outp")
                for bl in range(G):
                    for ci, (c0, n) in enumerate(CHUNKS):
                        ps = ppool.tile([MPAD, 512], F32, tag="ps")
                        for j in range(KW):
                            nc.tensor.matmul(
                                ps[:, 0:n],
                                lhsT=wt[:, j * MPAD:(j + 1) * MPAD],
                                rhs=t[:, bl * W + c0 + j:bl * W + c0 + j + n],
                                start=(j == 0),
                                stop=(j == KW - 1),
                            )
                        # Evict PSUM(+bias) on alternating engines: the
                        # f32 PSUM read is the per-engine throughput wall.
                        dst = ot[:, bl * WOUT + c0:bl * WOUT + c0 + n]
                        if ci == 0:
                            nc.vector.tensor_scalar_add(dst, ps[0:M, 0:n], bt)
                        else:
                            nc.scalar.activation(dst, ps[0:M, 0:n], ident,
                                                 bias=bt)
                nc.gpsimd.dma_start(out=out_d[:, b0 * WOUT:(b0 + G) * WOUT],
                                    in_=ot[:, 0:G * WOUT])
                b0 += G

    nc.compile()
    return nc


def _relayout_input(x):
    """[CIN,H,W] f32 -> [128, NB*W] bf16 with the roll + halo baked in."""
    xb = x.astype(NPBF16)
    rows = (D * np.arange(NB)[:, None] + np.arange(R)[None, :] - 1) % H
    g = xb[:, rows, :]                      # [c, b, q, w]
    return np.ascontiguousarray(g.transpose(2, 0, 1, 3)).reshape(
        R * CIN, NB * W)


def _pack_weights(filt):
    """wgt[q*CIN+c, j*MPAD + dx*COUT + co] = filt[co, c, q-dx, j]."""
    wm = np.zeros((R * CIN, KW * MPAD), np.float32)
    for j in range(KW):
        for q in range(R):
            for dx in range(D):
                i = q - dx
                if 0 <= i < KH:
                    for c in range(CIN):
                        wm[q * CIN + c, j * MPAD + dx * COUT:
                           j * MPAD + dx * COUT + COUT] = filt[:, c, i, j]
    return wm.astype(NPBF16)


def _prep_in_maps(inp, filt, bias):
    inp = np.asarray(inp, np.float32)
    filt = np.asarray(filt, np.float32)
    bias = np.asarray(bias, np.float32)
    wgt = _pack_weights(filt)
    bias112 = np.ascontiguousarray(np.tile(bias, D)[:, None])
    return [
        {"inp": _relayout_input(inp[n]), "wgt": wgt, "bias": bias112}
        for n in range(N_CORES)
    ]


def _unshard(res):
    outs = []
    for c in range(N_CORES):
        o = np.asarray(res.results[c]["out"]).astype(np.float32)
        o = o.reshape(D, COUT, NB, WOUT).transpose(1, 2, 0, 3)
        outs.append(o.reshape(COUT, HOUT, WOUT))
    return np.stack(outs, axis=0)


_CACHE = {}


def _get_nc():
    if "nc" not in _CACHE:
        _CACHE["nc"] = build_nc()
    return _CACHE["nc"]


def kernel(inp: np.ndarray, filt: np.ndarray, bias: np.ndarray) -> np.ndarray:
    nc = _get_nc()
    in_maps = _prep_in_maps(inp, filt, bias)
    res = run_bass_kernel_spmd(nc, in_maps, list(range(N_CORES)))
    return _unshard(res)


# revision 15
# speedup vs baseline: 1.0425x; 1.0425x over previous
"""Trainium2 Bass kernel: Conv2d [8,8,1024,1024] x [8,8,3,3] (+bias), with
the reference's roll-by-1 on H, VALID padding -> [8,8,1022,1022].

Strategy: data-parallel over the batch dim (1 image per NeuronCore, 8 cores).
The kernel is HBM-bandwidth bound (and the device HAM throttles HBM to ~50%
when all 8 cores stream), so the wire format is bf16 both ways and the host
pre-relayouts the input so every DMA moves big contiguous per-partition
spans:

  - Host packs the (rolled) input as inp_re[q*8+c, b*1024+w] =
    bf16(inp[c, (14b+q-1)%1024, w]): partition p = (row-in-block q, cin c),
    one 1024-col slab per conv block b (73 blocks x 14 output rows = 1022).
  - Per group of G=8..9 blocks, ONE input DMA moves [128, G*1024] with
    G*2KiB contiguous per partition; compute runs blocked matmuls on the
    tensor engine: lhsT [128,112] bf16 packs filt taps (column m = dx*8+co
    holds filt[co,c,q-dx,j]); the 3 W-taps are accumulating matmuls whose
    rhs is the same tile shifted by j. PSUM f32 [112,512] is evicted by
    DVE tensor_scalar_add(+bias) into a bf16 SBUF tile, and ONE output DMA
    per group writes [112, G*1022] contiguous.
  - Host unshards out_re[dx*8+co, b*1022+w] -> out[co, 14b+dx, w] and
    upcasts to f32.

bf16 error (inputs+weights+output quantized, f32 PSUM accumulate over the
72-term contraction) is ~0.3% of output scale, well under the 2e-2 gate.
"""

import os
import sys

for _p in ("/opt/trn_rl_repo",):
    if _p not in sys.path and os.path.isdir(_p):
        sys.path.insert(0, _p)

import ml_dtypes
import numpy as np

import concourse.bacc as bacc
import concourse.mybir as mybir
from concourse.bass_utils import run_bass_kernel_spmd
from concourse.tile import TileContext

F32 = mybir.dt.float32
BF16 = mybir.dt.bfloat16
NPBF16 = ml_dtypes.bfloat16

N_CORES = 8
CIN = 8
COUT = 8
KH = 3
KW = 3
H = W = 1024
HOUT = WOUT = 1022
D = 14            # output rows per block
R = D + 2         # input rows per block
M = COUT * D      # 112 matmul output columns (dx-major)
MPAD = 128        # lhsT padded to 128 cols: enables PE Fast Weight Load
NB = HOUT // D    # 73 blocks exactly
GROUPS = [2, 3] + [4] * 16 + [3, 1]   # sum = 73; small head + tail
GMAX = max(GROUPS)
CHUNKS = ((0, 512), (512, 510))  # PSUM bank = 512 f32


def build_nc(in_bufs: int = 8, out_bufs: int = 8, psum_bufs: int = 8):
    nc = bacc.Bacc("TRN2", target_bir_lowering=False, debug=False,
                   num_devices=N_CORES)
    inp_d = nc.dram_tensor("inp", [R * CIN, NB * W], BF16,
                           kind="ExternalInput")
    wgt_d = nc.dram_tensor("wgt", [R * CIN, KW * MPAD], BF16,
                           kind="ExternalInput")
    bias_d = nc.dram_tensor("bias", [M, 1], F32, kind="ExternalInput")
    out_d = nc.dram_tensor("out", [M, NB * WOUT], BF16,
                           kind="ExternalOutput")

    with TileContext(nc) as tc:
        with (
            tc.tile_pool(name="consts", bufs=1) as wpool,
            tc.tile_pool(name="inp", bufs=in_bufs) as ipool,
            tc.tile_pool(name="outp", bufs=out_bufs) as opool,
            tc.tile_pool(name="ps", bufs=psum_bufs, space="PSUM") as ppool,
        ):
            ident = mybir.ActivationFunctionType.Identity
            wt = wpool.tile([R * CIN, KW * MPAD], BF16, tag="wgt")
            nc.sync.dma_start(out=wt[:], in_=wgt_d[:])
            bt = wpool.tile([M, 1], F32, tag="bias")
            nc.sync.dma_start(out=bt[:], in_=bias_d[:])

            b0 = 0
            for gi, G in enumerate(GROUPS):
                t = ipool.tile([R * CIN, GMAX * W], BF16, tag="inp")
                nc.sync.dma_start(out=t[:, 0:G * W],
                                  in_=inp_d[:, b0 * W:(b0 + G) * W])
                ot = opool.tile([M, GMAX * WOUT], BF16, tag="outp")
                for bl in range(G):
                    for ci, (c0, n) in enumerate(CHUNKS):
                        ps = ppool.tile([MPAD, 512], F32, tag="ps")
                        for j in range(KW):
                            nc.tensor.matmul(
                                ps[:, 0:n],
                                lhsT=wt[:, j * MPAD:(j + 1) * MPAD],
                                rhs=t[:, bl * W + c0 + j:bl * W + c0 + j + n],
                                start=(j == 0),
                                stop=(j == KW - 1),
                            )
                        # Evict PSUM(+bias) on alternating engines: the
                        # f32 PSUM read is the per-engine throughput wall.
                        dst = ot[:, bl * WOUT + c0:bl * WOUT + c0 + n]
                        if ci == 0:
                            nc.vector.tensor_scalar_add(dst, ps[0:M, 0:n], bt)
                        else:
                            nc.scalar.activation(dst, ps[0:M, 0:n], ident,
                                                 bias=bt)
                nc.gpsimd.dma_start(out=out_d[:, b0 * WOUT:(b0 + G) * WOUT],
                                    in_=ot[:, 0:G * WOUT])
                b0 += G

    nc.compile()
    return nc


def _relayout_input(x):
    """[CIN,H,W] f32 -> [128, NB*W] bf16 with the roll + halo baked in."""
    xb = x.astype(NPBF16)
    rows = (D * np.arange(NB)[:, None] + np.arange(R)[None, :] - 1) % H
    g = xb[:, rows, :]                      # [c, b, q, w]
    return np.ascontiguousarray(g.transpose(2, 0, 1, 3)).reshape(
        R * CIN, NB * W)


def _pack_weights(filt):
    """wgt[q*CIN+c, j*MPAD + dx*COUT + co] = filt[co, c, q-dx, j]."""
    wm = np.zeros((R * CIN, KW * MPAD), np.float32)
    for j in range(KW):
        for q in range(R):
            for dx in range(D):
                i = q - dx
                if 0 <= i < KH:
                    for c in range(CIN):
                        wm[q * CIN + c, j * MPAD + dx * COUT:
                           j * MPAD + dx * COUT + COUT] = filt[:, c, i, j]
    return wm.astype(NPBF16)


def _prep_in_maps(inp, filt, bias):
    inp = np.asarray(inp, np.float32)
    filt = np.asarray(filt, np.float32)
    bias = np.asarray(bias, np.float32)
    wgt = _pack_weights(filt)
    bias112 = np.ascontiguousarray(np.tile(bias, D)[:, None])
    return [
        {"inp": _relayout_input(inp[n]), "wgt": wgt, "bias": bias112}
        for n in range(N_CORES)
    ]


def _unshard(res):
    outs = []
    for c in range(N_CORES):
        o = np.asarray(res.results[c]["out"]).astype(np.float32)
        o = o.reshape(D, COUT, NB, WOUT).transpose(1, 2, 0, 3)
        outs.append(o.reshape(COUT, HOUT, WOUT))
    return np.stack(outs, axis=0)


_CACHE = {}


def _get_nc():
    if "nc" not in _CACHE:
        _CACHE["nc"] = build_nc()
    return _CACHE["nc"]


def kernel(inp: np.ndarray, filt: np.ndarray, bias: np.ndarray) -> np.ndarray:
    nc = _get_nc()
    in_maps = _prep_in_maps(inp, filt, bias)
    res = run_bass_kernel_spmd(nc, in_maps, list(range(N_CORES)))
    return _unshard(res)
